# revision 33
# baseline (speedup 1.0000x reference)
"""ConvLSTM (B=4, T=8, C=HID=256, H=W=32, 3x3 SAME convs) on 8 TRN2 NeuronCores.

Sharding: data-parallel over batch (4) x spatial halves of H (2) = 8 cores,
zero inter-core communication. Each core computes its half's rows plus a
shrinking halo margin (23-t rows at step t); wrong values erode inward from
the un-owned edge at 1 row/step, leaving the owned 16 rows correct after
T=8 steps. Upper halves are row-flipped host-side (with dy-flipped kernels)
so all 8 cores run the same SPMD instruction stream.

Compute: 1D Winograd F(2,3) along W. Each 3x3 conv becomes 3(dy) x 4(u)
transform-point matmuls over 16 column-tiles -- 2/3 the PE columns of the
direct 9-tap method. All matmul data is bf16 (full PE rate at any moving
size; fp32 PSUM accumulation). The x-side input transform and both weight
transforms (G w: [w0, (w0+w1+w2)/2, (w0-w1+w2)/2, w2]) are precomputed on
the host. The h-side transform runs on DVE/GpSimd each step as 4 shifted
add/subs per ic-half: h is stored as even/odd column planes (he/ho) written
directly by the state update, so spatial h is never materialized. Output
transform (y0 = m0+m1+m2, y1 = m1-m2-m3) drains PSUM on DVE (y0) and
GpSimd (y1); gates run on the scalar engine (sigmoid/relu, bias fused).
"""
import numpy as np
import ml_dtypes
from contextlib import ExitStack

import concourse.bass as bass
import concourse.tile as tile
from concourse import bacc, mybir
from concourse.bass_utils import run_bass_kernel_spmd

BF16 = mybir.dt.bfloat16
F32 = mybir.dt.float32
AF = mybir.ActivationFunctionType
ALU = mybir.AluOpType

N_CORES = 8
T = 8
ROWS = 26           # V/he/ho buffer rows: p=0 is y=-1 (zero), p=1..25 = y=0..24
NB = 16             # column tiles (W=32 -> 16 tiles of 2 output cols)
VPL = 4 * ROWS * NB     # 1664 V elements per ic-half
HPL = ROWS * 17         # 442 he/ho elements per ic-half
R_LIST = [23 - t for t in range(T)]   # computed rows per step

_cache = {}


def _build_nc():
    nc = bacc.Bacc("TRN2", target_bir_lowering=False, debug=False,
                   num_devices=N_CORES)
    vx_d = nc.dram_tensor("vx", [T, 128, 2 * VPL], BF16, kind="ExternalInput").ap()
    wa_d = nc.dram_tensor("wa", [48, 128, 768], BF16, kind="ExternalInput").ap()
    wb_d = nc.dram_tensor("wb", [48, 128, 256], BF16, kind="ExternalInput").ap()
    b_d = nc.dram_tensor("bias", [128, 8], F32, kind="ExternalInput").ap()
    z_d = nc.dram_tensor("hz", [128, 2 * HPL], BF16, kind="ExternalInput").ap()
    out_d = nc.dram_tensor("hout", [2, 128, 512], F32, kind="ExternalOutput").ap()

    with tile.TileContext(nc) as tc, ExitStack() as ctx:
        wp = ctx.enter_context(tc.tile_pool(name="wp", bufs=1))
        vxp = ctx.enter_context(tc.tile_pool(name="vxp", bufs=2))
        hp = ctx.enter_context(tc.tile_pool(name="hp", bufs=1))
        cp = ctx.enter_context(tc.tile_pool(name="cp", bufs=1))
        bp = ctx.enter_context(tc.tile_pool(name="bp", bufs=1))
        yp = ctx.enter_context(tc.tile_pool(name="yp", bufs=4))
        sp = ctx.enter_context(tc.tile_pool(name="sp", bufs=3))
        gp = ctx.enter_context(tc.tile_pool(name="gp", bufs=10))
        crp = ctx.enter_context(tc.tile_pool(name="crp", bufs=3))
        pp = ctx.enter_context(tc.tile_pool(name="pp", bufs=2, space="PSUM"))

        bt = bp.tile([128, 8], F32, tag="bias")
        nc.sync.dma_start(bt[:], b_d[:])

        # weights: j = ((cv*2+it)*3+dy)*4+u. Stream order: x i/o/g gates,
        # h i/o/g, then the f-gate columns (x, h) last -- t=0 needs only
        # x-i/o/g and t=1's h-conv starts well before any f drain.
        was = [wp.tile([128, 768], BF16, tag=f"wa{j}", name=f"wa{j}")
               for j in range(48)]
        wbs = [wp.tile([128, 256], BF16, tag=f"wb{j}", name=f"wb{j}")
               for j in range(48)]
        for j in range(48):
            nc.sync.dma_start(was[j][:], wa_d[j])
        for j in range(48):
            nc.sync.dma_start(wbs[j][:], wb_d[j])

        def wslice(j, o):
            if o < 6:
                return was[j][:, o * 128:(o + 1) * 128]
            return wbs[j][:, (o - 6) * 128:(o - 5) * 128]

        # h state as even/odd column planes; col 0 of ho (h[-1]) and col 16
        # of he (h[32]) are permanent SAME-padding zeros, as is row p=0.
        he_t = hp.tile([128, 2 * HPL], BF16, tag="he")
        ho_t = hp.tile([128, 2 * HPL], BF16, tag="ho")
        he = he_t[:].rearrange("p (i r c) -> p i r c", i=2, r=ROWS, c=17)
        ho = ho_t[:].rearrange("p (i r c) -> p i r c", i=2, r=ROWS, c=17)

        # double-buffered so step t+1's transform overlaps step t's h-matmuls
        vh_views = []
        for k in range(2):
            vh_t = hp.tile([128, 2 * VPL], BF16, tag=f"vh{k}", name=f"vh{k}")
            vh_views.append(vh_t[:].rearrange("p (i u r c) -> p i u r c",
                                              i=2, u=4, r=ROWS, c=NB))

        c_t = cp.tile([128, 2 * 2 * 384], F32, tag="c")
        cv_ = c_t[:].rearrange("p (i v n) -> p i v n", i=2, v=2, n=384)
        nc.vector.memset(c_t[:], 0.0)

        # PE p-state warmup: the clock ramps 0.65->2.4GHz over ~3us of busy
        # time, and t0's 144 matmuls otherwise run ~30% slow (208 vs 158ns).
        # Dependency-free fp32 dummies (4 cycles/row) burn the ~13us DMA
        # window; t0 is DMA-paced so a small overrun costs nothing.
        wu = cp.tile([128, 640], F32, tag="wu")
        nc.vector.memset(wu[:], 0.0)
        wu_ps = pp.tile([128, 4 * 512], F32, tag="ps")
        for _ in range(10):
            nc.tensor.matmul(wu_ps[:, :512], wu[:, :128], wu[:, 128:640],
                             start=True, stop=True, skip_group_check=True)

        hof_t = cp.tile([128, 2 * 2 * 256], F32, tag="hof")
        hof = hof_t[:].rearrange("p (i v n) -> p i v n", i=2, v=2, n=256)

        # x V-planes + h zero-fill ride the gpsimd (SWDGE) queue so they
        # don't wait behind the 12.6MB weight stream on the sync queue.
        vx0 = vxp.tile([128, 2 * VPL], BF16, tag="vx")
        nc.gpsimd.dma_start(vx0[:], vx_d[0])
        nc.gpsimd.dma_start(he_t[:], z_d[:])
        nc.gpsimd.dma_start(ho_t[:], z_d[:])
        vx_tiles = [vx0]

        def emit_conv(ps, vsrc, cv, o, r, start_x, stop_h, its=(0, 1)):
            # 24 matmuls: it(2) x dy(3) x u(4), j-ascending = stream order
            n = r * NB
            for it in its:
                for dy in range(3):
                    for u in range(4):
                        j = ((cv * 2 + it) * 3 + dy) * 4 + u
                        nc.tensor.matmul(
                            ps[:, u, :n],
                            wslice(j, o),
                            vsrc[:, it, u, dy: dy + r, :],
                            start=(start_x and it == 0 and dy == 0),
                            stop=(stop_h and it == 1 and dy == 2),
                            skip_group_check=True)

        def drain(o, ps, r, gts):
            # tensor_tensor may read at most ONE input from PSUM: stage m1
            # to SBUF on the scalar engine, then chain 1-PSUM-operand ops.
            # y planes are packed [0:n]=y0, [n:2n]=y1 so the activation and
            # all state math run on contiguous 2D slices.
            n = r * NB
            yt = yp.tile([128, 2 * 368], F32, tag="y")
            s1 = sp.tile([128, 368], F32, tag="s1")
            nc.scalar.copy(s1[:, :n], ps[:, 1, :n])
            # GpSimd cannot access PSUM: all four y ops run on DVE, each
            # reading exactly one PSUM operand. y0 = m0+m1+m2, y1 = m1-m2-m3.
            nc.vector.tensor_sub(yt[:, 368:368 + n], s1[:, :n], ps[:, 2, :n])
            nc.vector.tensor_add(yt[:, 0:n], ps[:, 0, :n], s1[:, :n])
            nc.vector.tensor_add(yt[:, 0:n], yt[:, 0:n], ps[:, 2, :n])
            nc.vector.tensor_sub(yt[:, 368:368 + n], yt[:, 368:368 + n],
                                 ps[:, 3, :n])
            gt = gp.tile([128, 2 * 368], F32, tag="g")
            gts[o] = gt
            # octile order [i0 i1 o0 o1 g0 g1 f0 f1]: g -> relu, rest sigmoid
            func = AF.Relu if o in (4, 5) else AF.Sigmoid
            nc.scalar.activation(gt[:, 0:n], yt[:, 0:n], func,
                                 bias=bt[:, o:o + 1])
            nc.scalar.activation(gt[:, 368:368 + n], yt[:, 368:368 + n], func,
                                 bias=bt[:, o:o + 1])

        def emit_state(it, t, r, gts):
            n = r * NB
            gi, go, gg = gts[0 + it], gts[2 + it], gts[4 + it]
            cr = crp.tile([128, 2 * 368], F32, tag="cr")
            for v in range(2):
                f0 = v * 368
                cs = cv_[:, it, v, :n]
                giv, ggv, gov = (gi[:, f0:f0 + n], gg[:, f0:f0 + n],
                                 go[:, f0:f0 + n])
                if t == 0:
                    nc.gpsimd.tensor_mul(cs, giv, ggv)
                else:
                    gfv = gts[6 + it][:, f0:f0 + n]
                    nc.gpsimd.tensor_mul(ggv, giv, ggv)
                    nc.vector.tensor_mul(cs, gfv, cs)
                    nc.gpsimd.tensor_add(cs, cs, ggv)
                crv = cr[:, f0:f0 + n]
                nc.scalar.activation(crv, cs, AF.Relu)
                eng = nc.vector if v == 0 else nc.gpsimd
                if t == T - 1:
                    eng.tensor_mul(hof[:, it, v, :n], gov, crv)
                else:
                    dst = (he[:, it, 1:r + 1, 0:16] if v == 0
                           else ho[:, it, 1:r + 1, 1:17])
                    eng.tensor_mul(
                        dst,
                        gov.rearrange("p (r b) -> p r b", r=r, b=16),
                        crv.rearrange("p (r b) -> p r b", r=r, b=16))

        def emit_vh(vh, it, rn):
            # h input transform for the NEXT step: V rows 0..rn+1
            rr = rn + 2
            nc.gpsimd.tensor_sub(vh[:, it, 0, :rr, :],
                                 ho[:, it, :rr, 0:16], ho[:, it, :rr, 1:17])
            nc.vector.tensor_add(vh[:, it, 1, :rr, :],
                                 he[:, it, :rr, 0:16], ho[:, it, :rr, 1:17])
            nc.vector.tensor_sub(vh[:, it, 2, :rr, :],
                                 ho[:, it, :rr, 1:17], he[:, it, :rr, 0:16])
            nc.gpsimd.tensor_sub(vh[:, it, 3, :rr, :],
                                 he[:, it, :rr, 0:16], he[:, it, :rr, 1:17])

        # Octile order [0,2,4,6,...]: the it0-half gates (octiles 0,2,4,6)
        # finish mid-step, so state(it0) and the next step's V_h(it0)
        # transform (into the other vh buffer) overlap the remaining PE work
        # instead of serializing after the last drain.
        for t in range(T):
            r = R_LIST[t]
            vxt = vx_tiles[t]
            if t + 1 < T:
                # vx[1] must beat the weight stream -> gp queue; later
                # prefetches ride sync, keeping the gp compute queue clear
                nv = vxp.tile([128, 2 * VPL], BF16, tag="vx")
                (nc.gpsimd if t == 0 else nc.sync).dma_start(nv[:], vx_d[t + 1])
                vx_tiles.append(nv)
            vx = vxt[:].rearrange("p (i u r c) -> p i u r c",
                                  i=2, u=4, r=ROWS, c=NB)
            vh = vh_views[t % 2]
            vh_next = vh_views[(t + 1) % 2]
            r_next = R_LIST[t + 1] if t + 1 < T else 0

            gts = {}
            ps_tiles = {}

            def post_half(it):
                emit_state(it, t, r, gts)
                if t + 1 < T:
                    emit_vh(vh_next, it, r_next)
                elif it == 0:
                    nc.sync.dma_start(
                        out_d[0], hof[:, 0].rearrange("p v n -> p (v n)"))

            if t == 0:
                for i, o in enumerate([0, 2, 4, 1, 3, 5]):
                    ps_t = pp.tile([128, 4 * 512], F32, tag="ps")
                    ps = ps_t[:].rearrange("p (u n) -> p u n", u=4, n=512)
                    emit_conv(ps, vx, 0, o, r, start_x=True, stop_h=True)
                    drain(o, ps, r, gts)
                    if i == 2:
                        post_half(0)
                post_half(1)
            else:
                # h(0)'s it1-half waits on the previous step's tail chain
                # (drain(7) -> state(it1) -> vh(it1), ~7us). Hoisting x(2)
                # between h(0)'s two it-halves stretches the PE cover from
                # ~5.5us to ~9.2us so the chain is fully hidden.
                oseq = [0, 2, 4, 6, 1, 3, 5, 7]
                for i, o in enumerate(oseq):
                    if i == 1:
                        emit_conv(ps_tiles[0], vh, 1, 0, r,
                                  start_x=False, stop_h=False, its=(0,))
                    elif i >= 2:
                        po = oseq[i - 1]
                        emit_conv(ps_tiles[po], vh, 1, po, r,
                                  start_x=False, stop_h=True)
                        drain(po, ps_tiles[po], r, gts)
                        if i == 5:
                            post_half(0)
                    ps_t = pp.tile([128, 4 * 512], F32, tag="ps")
                    ps = ps_t[:].rearrange("p (u n) -> p u n", u=4, n=512)
                    ps_tiles[o] = ps
                    emit_conv(ps, vx, 0, o, r, start_x=True, stop_h=False)
                    if i == 1:
                        emit_conv(ps_tiles[0], vh, 1, 0, r,
                                  start_x=False, stop_h=True, its=(1,))
                        drain(0, ps_tiles[0], r, gts)
                emit_conv(ps_tiles[7], vh, 1, 7, r, start_x=False, stop_h=True)
                drain(7, ps_tiles[7], r, gts)
                post_half(1)

        nc.sync.dma_start(out_d[1], hof[:, 1].rearrange("p v n -> p (v n)"))

    nc.compile()
    return nc


GATE_PERM = [0, 2, 3, 1]  # reorder [i, f, o, g] -> [i, o, g, f]
G1 = np.array([[1, 0, 0], [.5, .5, .5], [.5, -.5, .5], [0, 0, 1]], np.float32)


def _bf16(a):
    return np.ascontiguousarray(a, dtype=np.float32).astype(ml_dtypes.bfloat16)


def _prep_weights(wx, wh, flip):
    ws = np.stack([np.asarray(wx), np.asarray(wh)])  # [2, 1024, 256, 3, 3]
    if flip:
        ws = ws[:, :, :, ::-1, :]
    wt = np.einsum('ud,covyd->covyu', G1, ws.astype(np.float32))
    # [cv, gate, ht, ch, it, ic, dy, u] -> [cv, it, dy, u, ic, gate, ht, ch]
    wt = wt.reshape(2, 4, 2, 128, 2, 128, 3, 4)[:, GATE_PERM]
    wt = wt.transpose(0, 4, 6, 7, 5, 1, 2, 3).reshape(48, 128, 1024)
    return (_bf16(np.ascontiguousarray(wt[:, :, :768])),
            _bf16(np.ascontiguousarray(wt[:, :, 768:])))


def _prep_x(xb, flip):
    # xb: [T, 256, 32, 32] for one batch element -> V planes [T, 128, 2*VPL]
    xc = np.asarray(xb, dtype=np.float32)
    if flip:
        xc = xc[:, :, ::-1, :]
    xp = np.zeros((T, 256, ROWS, 34), dtype=np.float32)
    xp[:, :, 1:26, 1:33] = xc[:, :, 0:25, :]
    v = np.stack([
        xp[..., 0:32:2] - xp[..., 2:34:2],
        xp[..., 1:33:2] + xp[..., 2:34:2],
        xp[..., 2:34:2] - xp[..., 1:33:2],
        xp[..., 1:33:2] - xp[..., 3:34:2],
    ], axis=2)                                   # [T, 256, 4, 26, 16]
    v = v.reshape(T, 2, 128, 4, ROWS, NB).transpose(0, 2, 1, 3, 4, 5)
    return _bf16(np.ascontiguousarray(v).reshape(T, 128, 2 * VPL))


def kernel(x, wx, wh, bh):
    x = np.asarray(x, dtype=np.float32)
    B = x.shape[0]
    bias = np.ascontiguousarray(
        np.asarray(bh, dtype=np.float32).reshape(4, 2, 128)[GATE_PERM]
        .transpose(2, 0, 1).reshape(128, 8))

    w_lo = _prep_weights(wx, wh, flip=False)
    w_hi = _prep_weights(wx, wh, flip=True)

    in_maps = []
    for c in range(N_CORES):
        b, half = c // 2, c % 2
        wa, wb = w_hi if half else w_lo
        in_maps.append({
            "vx": _prep_x(x[b], flip=bool(half)),
            "wa": wa,
            "wb": wb,
            "bias": bias,
            "hz": np.zeros((128, 2 * HPL), dtype=ml_dtypes.bfloat16),
        })

    if "nc" not in _cache:
        _cache["nc"] = _build_nc()
    nc = _cache["nc"]

    res = run_bass_kernel_spmd(nc, in_maps, core_ids=list(range(N_CORES)))
    _cache["last_results"] = res

    out = np.zeros((B, 256, 32, 32), dtype=np.float32)
    for c in range(N_CORES):
        b, half = c // 2, c % 2
        arr = np.asarray(res.results[c]["hout"], dtype=np.float32)
        arr = arr.reshape(2, 128, 2, 16, 16)     # [it, p, v, y, b]
        h = np.zeros((2, 128, 16, 32), dtype=np.float32)
        h[:, :, :, 0::2] = arr[:, :, 0]
        h[:, :, :, 1::2] = arr[:, :, 1]
        h = h.reshape(256, 16, 32)
        if half:
            out[b, :, 16:32, :] = h[:, ::-1, :]
        else:
            out[b, :, 0:16, :] = h
    return out


# revision 37
# speedup vs baseline: 1.0044x; 1.0044x over previous
"""ConvLSTM (B=4, T=8, C=HID=256, H=W=32, 3x3 SAME convs) on 8 TRN2 NeuronCores.

Sharding: data-parallel over batch (4) x spatial halves of H (2) = 8 cores,
zero inter-core communication. Each core computes its half's rows plus a
shrinking halo margin (23-t rows at step t); wrong values erode inward from
the un-owned edge at 1 row/step, leaving the owned 16 rows correct after
T=8 steps. Upper halves are row-flipped host-side (with dy-flipped kernels)
so all 8 cores run the same SPMD instruction stream.

Compute: 1D Winograd F(2,3) along W. Each 3x3 conv becomes 3(dy) x 4(u)
transform-point matmuls over 16 column-tiles -- 2/3 the PE columns of the
direct 9-tap method. All matmul data is bf16 (full PE rate at any moving
size; fp32 PSUM accumulation). The x-side input transform and both weight
transforms (G w: [w0, (w0+w1+w2)/2, (w0-w1+w2)/2, w2]) are precomputed on
the host. The h-side transform runs on DVE/GpSimd each step as 4 shifted
add/subs per ic-half: h is stored as even/odd column planes (he/ho) written
directly by the state update, so spatial h is never materialized. Output
transform (y0 = m0+m1+m2, y1 = m1-m2-m3) drains PSUM on DVE (y0) and
GpSimd (y1); gates run on the scalar engine (sigmoid/relu, bias fused).
"""
import numpy as np
import ml_dtypes
from contextlib import ExitStack

import concourse.bass as bass
import concourse.tile as tile
from concourse import bacc, mybir
from concourse.bass_utils import run_bass_kernel_spmd

BF16 = mybir.dt.bfloat16
F32 = mybir.dt.float32
AF = mybir.ActivationFunctionType
ALU = mybir.AluOpType

N_CORES = 8
T = 8
ROWS = 26           # V/he/ho buffer rows: p=0 is y=-1 (zero), p=1..25 = y=0..24
NB = 16             # column tiles (W=32 -> 16 tiles of 2 output cols)
VPL = 4 * ROWS * NB     # 1664 V elements per ic-half
HPL = ROWS * 17         # 442 he/ho elements per ic-half
R_LIST = [23 - t for t in range(T)]   # computed rows per step

_cache = {}


def _build_nc():
    nc = bacc.Bacc("TRN2", target_bir_lowering=False, debug=False,
                   num_devices=N_CORES)
    vx_d = nc.dram_tensor("vx", [T, 128, 2 * VPL], BF16, kind="ExternalInput").ap()
    wa_d = nc.dram_tensor("wa", [48, 128, 768], BF16, kind="ExternalInput").ap()
    wb_d = nc.dram_tensor("wb", [48, 128, 256], BF16, kind="ExternalInput").ap()
    b_d = nc.dram_tensor("bias", [128, 8], F32, kind="ExternalInput").ap()
    z_d = nc.dram_tensor("hz", [128, 2 * HPL], BF16, kind="ExternalInput").ap()
    out_d = nc.dram_tensor("hout", [2, 128, 512], F32, kind="ExternalOutput").ap()

    with tile.TileContext(nc) as tc, ExitStack() as ctx:
        wp = ctx.enter_context(tc.tile_pool(name="wp", bufs=1))
        vxp = ctx.enter_context(tc.tile_pool(name="vxp", bufs=2))
        hp = ctx.enter_context(tc.tile_pool(name="hp", bufs=1))
        cp = ctx.enter_context(tc.tile_pool(name="cp", bufs=1))
        bp = ctx.enter_context(tc.tile_pool(name="bp", bufs=1))
        yp = ctx.enter_context(tc.tile_pool(name="yp", bufs=4))
        sp = ctx.enter_context(tc.tile_pool(name="sp", bufs=3))
        gp = ctx.enter_context(tc.tile_pool(name="gp", bufs=10))
        crp = ctx.enter_context(tc.tile_pool(name="crp", bufs=3))
        pp = ctx.enter_context(tc.tile_pool(name="pp", bufs=2, space="PSUM"))

        bt = bp.tile([128, 8], F32, tag="bias")
        nc.sync.dma_start(bt[:], b_d[:])

        # weights: j = ((cv*2+it)*3+dy)*4+u. Stream order: x i/o/g gates,
        # h i/o/g, then the f-gate columns (x, h) last -- t=0 needs only
        # x-i/o/g and t=1's h-conv starts well before any f drain.
        was = [wp.tile([128, 768], BF16, tag=f"wa{j}", name=f"wa{j}")
               for j in range(48)]
        wbs = [wp.tile([128, 256], BF16, tag=f"wb{j}", name=f"wb{j}")
               for j in range(48)]
        for j in range(48):
            nc.sync.dma_start(was[j][:], wa_d[j])
        for j in range(48):
            nc.sync.dma_start(wbs[j][:], wb_d[j])

        def wslice(j, o):
            if o < 6:
                return was[j][:, o * 128:(o + 1) * 128]
            return wbs[j][:, (o - 6) * 128:(o - 5) * 128]

        # h state as even/odd column planes; col 0 of ho (h[-1]) and col 16
        # of he (h[32]) are permanent SAME-padding zeros, as is row p=0.
        he_t = hp.tile([128, 2 * HPL], BF16, tag="he")
        ho_t = hp.tile([128, 2 * HPL], BF16, tag="ho")
        he = he_t[:].rearrange("p (i r c) -> p i r c", i=2, r=ROWS, c=17)
        ho = ho_t[:].rearrange("p (i r c) -> p i r c", i=2, r=ROWS, c=17)

        # double-buffered so step t+1's transform overlaps step t's h-matmuls
        vh_views = []
        for k in range(2):
            vh_t = hp.tile([128, 2 * VPL], BF16, tag=f"vh{k}", name=f"vh{k}")
            vh_views.append(vh_t[:].rearrange("p (i u r c) -> p i u r c",
                                              i=2, u=4, r=ROWS, c=NB))

        c_t = cp.tile([128, 2 * 2 * 384], F32, tag="c")
        cv_ = c_t[:].rearrange("p (i v n) -> p i v n", i=2, v=2, n=384)
        nc.vector.memset(c_t[:], 0.0)

        # PE p-state warmup: the clock ramps 0.65->2.4GHz over ~3us of busy
        # time, and t0's 144 matmuls otherwise run ~30% slow (208 vs 158ns).
        # Dependency-free fp32 dummies (4 cycles/row) burn the ~13us DMA
        # window; t0 is DMA-paced so a small overrun costs nothing.
        wu = cp.tile([128, 640], F32, tag="wu")
        nc.vector.memset(wu[:], 0.0)
        wu_ps = pp.tile([128, 4 * 512], F32, tag="ps")
        for _ in range(10):
            nc.tensor.matmul(wu_ps[:, :512], wu[:, :128], wu[:, 128:640],
                             start=True, stop=True, skip_group_check=True)

        hof_t = cp.tile([128, 2 * 2 * 256], F32, tag="hof")
        hof = hof_t[:].rearrange("p (i v n) -> p i v n", i=2, v=2, n=256)

        # x V-planes + h zero-fill ride the gpsimd (SWDGE) queue so they
        # don't wait behind the 12.6MB weight stream on the sync queue.
        vx0 = vxp.tile([128, 2 * VPL], BF16, tag="vx")
        nc.gpsimd.dma_start(vx0[:], vx_d[0])
        nc.gpsimd.dma_start(he_t[:], z_d[:])
        nc.gpsimd.dma_start(ho_t[:], z_d[:])
        vx_tiles = [vx0]

        def emit_conv(ps, vsrc, cv, o, r, start_x, stop_h, its=(0, 1)):
            # 24 matmuls: it(2) x dy(3) x u(4), j-ascending = stream order
            n = r * NB
            # h-conv runs u=3 first: its V3 plane (he-only) is produced
            # mid-state, ahead of the ho-dependent planes, at step start
            uord = (3, 0, 1, 2) if cv == 1 else (0, 1, 2, 3)
            for it in its:
                for dy in range(3):
                    for u in uord:
                        j = ((cv * 2 + it) * 3 + dy) * 4 + u
                        nc.tensor.matmul(
                            ps[:, u, :n],
                            wslice(j, o),
                            vsrc[:, it, u, dy: dy + r, :],
                            start=(start_x and it == 0 and dy == 0),
                            stop=(stop_h and it == 1 and dy == 2),
                            skip_group_check=True)

        def drain(o, ps, r, gts):
            # tensor_tensor may read at most ONE input from PSUM: stage m1
            # to SBUF on the scalar engine, then chain 1-PSUM-operand ops.
            # y planes are packed [0:n]=y0, [n:2n]=y1 so the activation and
            # all state math run on contiguous 2D slices.
            n = r * NB
            yt = yp.tile([128, 2 * 368], F32, tag="y")
            s1 = sp.tile([128, 368], F32, tag="s1")
            nc.scalar.copy(s1[:, :n], ps[:, 1, :n])
            # GpSimd cannot access PSUM: all four y ops run on DVE, each
            # reading exactly one PSUM operand. y0 = m0+m1+m2, y1 = m1-m2-m3.
            nc.vector.tensor_sub(yt[:, 368:368 + n], s1[:, :n], ps[:, 2, :n])
            nc.vector.tensor_add(yt[:, 0:n], ps[:, 0, :n], s1[:, :n])
            nc.vector.tensor_add(yt[:, 0:n], yt[:, 0:n], ps[:, 2, :n])
            nc.vector.tensor_sub(yt[:, 368:368 + n], yt[:, 368:368 + n],
                                 ps[:, 3, :n])
            gt = gp.tile([128, 2 * 368], F32, tag="g")
            gts[o] = gt
            # octile order [i0 i1 o0 o1 g0 g1 f0 f1]: g -> relu, rest sigmoid
            func = AF.Relu if o in (4, 5) else AF.Sigmoid
            nc.scalar.activation(gt[:, 0:n], yt[:, 0:n], func,
                                 bias=bt[:, o:o + 1])
            nc.scalar.activation(gt[:, 368:368 + n], yt[:, 368:368 + n], func,
                                 bias=bt[:, o:o + 1])

        def emit_state(it, t, r, gts, vh_mid=None, rn=0):
            n = r * NB
            gi, go, gg = gts[0 + it], gts[2 + it], gts[4 + it]
            cr = crp.tile([128, 2 * 368], F32, tag="cr")
            for v in range(2):
                if v == 1 and vh_mid is not None:
                    # V3 depends only on he (the v0 half just written):
                    # emit it on gp BEFORE the v1 ops so the next step's
                    # u=3 h-matmuls unblock ~2.5us earlier
                    rr = rn + 2
                    nc.gpsimd.tensor_sub(vh_mid[:, it, 3, :rr, :],
                                         he[:, it, :rr, 0:16],
                                         he[:, it, :rr, 1:17])
                f0 = v * 368
                cs = cv_[:, it, v, :n]
                giv, ggv, gov = (gi[:, f0:f0 + n], gg[:, f0:f0 + n],
                                 go[:, f0:f0 + n])
                if t == 0:
                    nc.gpsimd.tensor_mul(cs, giv, ggv)
                else:
                    gfv = gts[6 + it][:, f0:f0 + n]
                    nc.gpsimd.tensor_mul(ggv, giv, ggv)
                    nc.vector.tensor_mul(cs, gfv, cs)
                    nc.gpsimd.tensor_add(cs, cs, ggv)
                crv = cr[:, f0:f0 + n]
                nc.scalar.activation(crv, cs, AF.Relu)
                eng = nc.vector if v == 0 else nc.gpsimd
                if t == T - 1:
                    eng.tensor_mul(hof[:, it, v, :n], gov, crv)
                else:
                    dst = (he[:, it, 1:r + 1, 0:16] if v == 0
                           else ho[:, it, 1:r + 1, 1:17])
                    eng.tensor_mul(
                        dst,
                        gov.rearrange("p (r b) -> p r b", r=r, b=16),
                        crv.rearrange("p (r b) -> p r b", r=r, b=16))

        def emit_vh(vh, it, rn):
            # h input transform for the NEXT step: V rows 0..rn+1.
            # V3 was already emitted mid-state; V0 (first ho-dependent
            # plane the matmuls need) goes right after the ho write.
            rr = rn + 2
            nc.gpsimd.tensor_sub(vh[:, it, 0, :rr, :],
                                 ho[:, it, :rr, 0:16], ho[:, it, :rr, 1:17])
            nc.vector.tensor_add(vh[:, it, 1, :rr, :],
                                 he[:, it, :rr, 0:16], ho[:, it, :rr, 1:17])
            nc.vector.tensor_sub(vh[:, it, 2, :rr, :],
                                 ho[:, it, :rr, 1:17], he[:, it, :rr, 0:16])

        # Octile order [0,2,4,6,...]: the it0-half gates (octiles 0,2,4,6)
        # finish mid-step, so state(it0) and the next step's V_h(it0)
        # transform (into the other vh buffer) overlap the remaining PE work
        # instead of serializing after the last drain.
        for t in range(T):
            r = R_LIST[t]
            vxt = vx_tiles[t]
            if t + 1 < T:
                # vx[1] must beat the weight stream -> gp queue; later
                # prefetches ride sync, keeping the gp compute queue clear
                nv = vxp.tile([128, 2 * VPL], BF16, tag="vx")
                (nc.gpsimd if t == 0 else nc.sync).dma_start(nv[:], vx_d[t + 1])
                vx_tiles.append(nv)
            vx = vxt[:].rearrange("p (i u r c) -> p i u r c",
                                  i=2, u=4, r=ROWS, c=NB)
            vh = vh_views[t % 2]
            vh_next = vh_views[(t + 1) % 2]
            r_next = R_LIST[t + 1] if t + 1 < T else 0

            gts = {}
            ps_tiles = {}

            def post_half(it):
                nxt = t + 1 < T
                emit_state(it, t, r, gts,
                           vh_mid=vh_next if nxt else None, rn=r_next)
                if nxt:
                    emit_vh(vh_next, it, r_next)
                elif it == 0:
                    nc.sync.dma_start(
                        out_d[0], hof[:, 0].rearrange("p v n -> p (v n)"))

            if t == 0:
                for i, o in enumerate([0, 2, 4, 1, 3, 5]):
                    ps_t = pp.tile([128, 4 * 512], F32, tag="ps")
                    ps = ps_t[:].rearrange("p (u n) -> p u n", u=4, n=512)
                    emit_conv(ps, vx, 0, o, r, start_x=True, stop_h=True)
                    drain(o, ps, r, gts)
                    if i == 2:
                        post_half(0)
                post_half(1)
            else:
                # h(0)'s it1-half waits on the previous step's tail chain
                # (drain(7) -> state(it1) -> vh(it1), ~7us). Hoisting x(2)
                # between h(0)'s two it-halves stretches the PE cover from
                # ~5.5us to ~9.2us so the chain is fully hidden.
                oseq = [0, 2, 4, 6, 1, 3, 5, 7]
                for i, o in enumerate(oseq):
                    if i == 1:
                        emit_conv(ps_tiles[0], vh, 1, 0, r,
                                  start_x=False, stop_h=False, its=(0,))
                    elif i >= 2:
                        po = oseq[i - 1]
                        emit_conv(ps_tiles[po], vh, 1, po, r,
                                  start_x=False, stop_h=True)
                        drain(po, ps_tiles[po], r, gts)
                        if i == 5:
                            post_half(0)
                    ps_t = pp.tile([128, 4 * 512], F32, tag="ps")
                    ps = ps_t[:].rearrange("p (u n) -> p u n", u=4, n=512)
                    ps_tiles[o] = ps
                    emit_conv(ps, vx, 0, o, r, start_x=True, stop_h=False)
                    if i == 1:
                        emit_conv(ps_tiles[0], vh, 1, 0, r,
                                  start_x=False, stop_h=True, its=(1,))
                        drain(0, ps_tiles[0], r, gts)
                emit_conv(ps_tiles[7], vh, 1, 7, r, start_x=False, stop_h=True)
                drain(7, ps_tiles[7], r, gts)
                post_half(1)

        nc.sync.dma_start(out_d[1], hof[:, 1].rearrange("p v n -> p (v n)"))

    nc.compile()
    return nc


GATE_PERM = [0, 2, 3, 1]  # reorder [i, f, o, g] -> [i, o, g, f]
G1 = np.array([[1, 0, 0], [.5, .5, .5], [.5, -.5, .5], [0, 0, 1]], np.float32)


def _bf16(a):
    return np.ascontiguousarray(a, dtype=np.float32).astype(ml_dtypes.bfloat16)


def _prep_weights(wx, wh, flip):
    ws = np.stack([np.asarray(wx), np.asarray(wh)])  # [2, 1024, 256, 3, 3]
    if flip:
        ws = ws[:, :, :, ::-1, :]
    wt = np.einsum('ud,covyd->covyu', G1, ws.astype(np.float32))
    # [cv, gate, ht, ch, it, ic, dy, u] -> [cv, it, dy, u, ic, gate, ht, ch]
    wt = wt.reshape(2, 4, 2, 128, 2, 128, 3, 4)[:, GATE_PERM]
    wt = wt.transpose(0, 4, 6, 7, 5, 1, 2, 3).reshape(48, 128, 1024)
    return (_bf16(np.ascontiguousarray(wt[:, :, :768])),
            _bf16(np.ascontiguousarray(wt[:, :, 768:])))


def _prep_x(xb, flip):
    # xb: [T, 256, 32, 32] for one batch element -> V planes [T, 128, 2*VPL]
    xc = np.asarray(xb, dtype=np.float32)
    if flip:
        xc = xc[:, :, ::-1, :]
    xp = np.zeros((T, 256, ROWS, 34), dtype=np.float32)
    xp[:, :, 1:26, 1:33] = xc[:, :, 0:25, :]
    v = np.stack([
        xp[..., 0:32:2] - xp[..., 2:34:2],
        xp[..., 1:33:2] + xp[..., 2:34:2],
        xp[..., 2:34:2] - xp[..., 1:33:2],
        xp[..., 1:33:2] - xp[..., 3:34:2],
    ], axis=2)                                   # [T, 256, 4, 26, 16]
    v = v.reshape(T, 2, 128, 4, ROWS, NB).transpose(0, 2, 1, 3, 4, 5)
    return _bf16(np.ascontiguousarray(v).reshape(T, 128, 2 * VPL))


def kernel(x, wx, wh, bh):
    x = np.asarray(x, dtype=np.float32)
    B = x.shape[0]
    bias = np.ascontiguousarray(
        np.asarray(bh, dtype=np.float32).reshape(4, 2, 128)[GATE_PERM]
        .transpose(2, 0, 1).reshape(128, 8))

    w_lo = _prep_weights(wx, wh, flip=False)
    w_hi = _prep_weights(wx, wh, flip=True)

    in_maps = []
    for c in range(N_CORES):
        b, half = c // 2, c % 2
        wa, wb = w_hi if half else w_lo
        in_maps.append({
            "vx": _prep_x(x[b], flip=bool(half)),
            "wa": wa,
            "wb": wb,
            "bias": bias,
            "hz": np.zeros((128, 2 * HPL), dtype=ml_dtypes.bfloat16),
        })

    if "nc" not in _cache:
        _cache["nc"] = _build_nc()
    nc = _cache["nc"]

    res = run_bass_kernel_spmd(nc, in_maps, core_ids=list(range(N_CORES)))
    _cache["last_results"] = res

    out = np.zeros((B, 256, 32, 32), dtype=np.float32)
    for c in range(N_CORES):
        b, half = c // 2, c % 2
        arr = np.asarray(res.results[c]["hout"], dtype=np.float32)
        arr = arr.reshape(2, 128, 2, 16, 16)     # [it, p, v, y, b]
        h = np.zeros((2, 128, 16, 32), dtype=np.float32)
        h[:, :, :, 0::2] = arr[:, :, 0]
        h[:, :, :, 1::2] = arr[:, :, 1]
        h = h.reshape(256, 16, 32)
        if half:
            out[b, :, 16:32, :] = h[:, ::-1, :]
        else:
            out[b, :, 0:16, :] = h
    return out
